# revision 5
# baseline (speedup 1.0000x reference)
"""Trainium2 Bass kernel for NodeUpdateNetwork-style GNN message passing.

out = relu(BN((x + ((sim - dsim) @ x) / N) @ W.T))  with sync-BN over (B, N).

Sharding: data-parallel over batch across 8 NeuronCores (2 batches/core);
W/gamma/beta replicated; BN statistics all-reduced across cores in-kernel.

The on-chip pipeline keeps the feature dimension on partitions ("transposed
space") so that BN reduces run along the free axis and the BN+ReLU apply is a
single per-partition scalar-engine activation:
  - stream sim/dsim row-stripes [128, N] fp32 (contiguous HBM reads)
  - DVE: diff = sim - dsim (bf16 out)
  - PE: transpose 128x128 diff tiles (identity matmul, bf16)
  - PE: aggT[f, i] += (x/N)[j, f]^T-contracted with diffT[j, i]
  - yT = aggT + xT ; zT = W @ yT (fp32) ; BN stats; AllReduce; apply; untranspose
"""

import sys

if "/opt/trn_rl_repo" not in sys.path:
    sys.path.insert(0, "/opt/trn_rl_repo")

import numpy as np
import ml_dtypes

import concourse.bacc as bacc
import concourse.mybir as mybir
import concourse.tile as tile
from concourse.bass_utils import run_bass_kernel_spmd

N_CORES = 8
B, N, F = 16, 2048, 64
B_PC = B // N_CORES
BN_EPS = 1e-5
BF16 = mybir.dt.bfloat16
F32 = mybir.dt.float32


def build_nc(n_cores=N_CORES, b_pc=B_PC, n=N, f=F, b_total=None):
    """Build the per-core Bass program (same program on every core)."""
    assert f == 64
    if b_total is None:
        b_total = n_cores * b_pc
    NT = n // 128                      # number of 128-wide j tiles
    IB = min(4, NT)                    # i-blocks (128 rows) per group
    GW = IB * 128                      # group width along i (<= 512)
    NG = n // GW                       # groups per batch
    inv_count = 1.0 / (b_total * n)

    nc = bacc.Bacc(
        "TRN2", target_bir_lowering=False, debug=False, num_devices=n_cores
    )

    edge = nc.dram_tensor("edge", [b_pc, 2, n, n], F32, kind="ExternalInput").ap()
    xt = nc.dram_tensor("xt", [b_pc, f, n], F32, kind="ExternalInput").ap()
    xn = nc.dram_tensor("xn", [b_pc, n, f], BF16, kind="ExternalInput").ap()
    wt = nc.dram_tensor("wt", [f, f], F32, kind="ExternalInput").ap()
    gamma = nc.dram_tensor("gamma", [f, 1], F32, kind="ExternalInput").ap()
    beta = nc.dram_tensor("beta", [f, 1], F32, kind="ExternalInput").ap()
    i128 = nc.dram_tensor("i128", [128, 128], BF16, kind="ExternalInput").ap()
    i64 = nc.dram_tensor("i64", [f, f], F32, kind="ExternalInput").ap()
    out = nc.dram_tensor("out", [b_pc, n, f], F32, kind="ExternalOutput").ap()

    with tile.TileContext(nc) as tc:
        with (
            tc.tile_pool(name="const", bufs=1) as cpool,
            tc.tile_pool(name="io", bufs=2) as iopool,
            tc.tile_pool(name="zq", bufs=b_pc) as zqpool,
            tc.tile_pool(name="stream", bufs=3) as spool,
            tc.tile_pool(name="blk", bufs=2) as bpool,
            tc.tile_pool(name="psum", bufs=2, space="PSUM") as ppool,
            tc.tile_pool(name="dram", bufs=1, space="DRAM") as dpool,
        ):
            # --- constants ---
            i128_sb = cpool.tile([128, 128], BF16)
            nc.sync.dma_start(i128_sb[:], i128[:])
            i64_sb = cpool.tile([f, f], F32)
            nc.sync.dma_start(i64_sb[:], i64[:])
            wt_sb = cpool.tile([f, f], F32)
            nc.sync.dma_start(wt_sb[:], wt[:])
            gamma_sb = cpool.tile([f, 1], F32)
            nc.sync.dma_start(gamma_sb[:], gamma[:])
            beta_sb = cpool.tile([f, 1], F32)
            nc.sync.dma_start(beta_sb[:], beta[:])
            stats_sb = cpool.tile([f, b_pc * NG, 2], F32)
            sc_sb = cpool.tile([f, 12], F32)

            zq_tiles = []
            for b in range(b_pc):
                # --- per-batch node features ---
                xt_sb = iopool.tile([f, n], F32, tag="xt")
                nc.sync.dma_start(xt_sb[:], xt[b])
                xn_sb = iopool.tile([128, NT, f], BF16, tag="xn")
                nc.sync.dma_start(
                    xn_sb[:], xn[b].rearrange("(t p) f -> p t f", p=128)
                )
                zq_sb = zqpool.tile([f, n], F32, tag="zq")
                zq_tiles.append(zq_sb)

                for g in range(NG):
                    # --- load group stripes and subtract ---
                    diff_all = bpool.tile([128, IB, n], BF16, tag="diff")
                    for ib in range(IB):
                        i0 = g * GW + ib * 128
                        sim_sb = spool.tile([128, n], F32, tag="sim")
                        nc.sync.dma_start(sim_sb[:], edge[b, 0, i0 : i0 + 128, :])
                        dsim_sb = spool.tile([128, n], F32, tag="dsim")
                        nc.sync.dma_start(dsim_sb[:], edge[b, 1, i0 : i0 + 128, :])
                        nc.vector.tensor_sub(diff_all[:, ib, :], sim_sb[:], dsim_sb[:])

                    # --- transpose diff tiles: dT[j, i] = diff[i, j] ---
                    dT_all = bpool.tile([128, NT, GW], BF16, tag="dT")
                    for jt in range(NT):
                        tpsum = ppool.tile([128, GW], BF16, tag="tpsum")
                        for ib in range(IB):
                            nc.tensor.transpose(
                                tpsum[:, ib * 128 : (ib + 1) * 128],
                                diff_all[:, ib, jt * 128 : (jt + 1) * 128],
                                i128_sb[:],
                            )
                        if jt % 2 == 0:
                            nc.vector.tensor_copy(dT_all[:, jt, :], tpsum[:])
                        else:
                            nc.scalar.copy(dT_all[:, jt, :], tpsum[:])

                    # --- aggT[f, i] = sum_j (x/N)[j, f] * diff[i, j] ---
                    aggT = ppool.tile([f, GW], F32, tag="agg")
                    for jt in range(NT):
                        nc.tensor.matmul(
                            aggT[:],
                            xn_sb[:, jt, :],
                            dT_all[:, jt, :],
                            start=(jt == 0),
                            stop=(jt == NT - 1),
                        )

                    # --- yT = aggT + xT ; zT = W @ yT ---
                    yT_sb = bpool.tile([f, GW], F32, tag="yT")
                    nc.vector.tensor_add(
                        yT_sb[:], aggT[:], xt_sb[:, g * GW : (g + 1) * GW]
                    )
                    zT = ppool.tile([f, GW], F32, tag="zT")
                    nc.tensor.matmul(zT[:], wt_sb[:], yT_sb[:], start=True, stop=True)

                    # stash z and accumulate BN partial sums
                    nc.scalar.copy(zq_sb[:, g * GW : (g + 1) * GW], zT[:])
                    gi = b * NG + g
                    nc.vector.reduce_sum(
                        stats_sb[:, gi, 0:1], zT[:], axis=mybir.AxisListType.X
                    )
                    sq_sb = bpool.tile([f, GW], F32, tag="sq")
                    nc.scalar.activation(
                        sq_sb[:],
                        zT[:],
                        mybir.ActivationFunctionType.Square,
                        accum_out=stats_sb[:, gi, 1:2],
                    )

            # --- local stats -> global stats (sync-BN all-reduce) ---
            stats_loc = cpool.tile([f, 2], F32)
            nc.vector.reduce_sum(
                stats_loc[:],
                stats_sb[:].rearrange("p g s -> p s g"),
                axis=mybir.AxisListType.X,
            )
            cc_in = dpool.tile([f, 2], F32)
            cc_out = dpool.tile([f, 2], F32)
            nc.sync.dma_start(cc_in[:], stats_loc[:])
            nc.gpsimd.collective_compute(
                "AllReduce",
                mybir.AluOpType.add,
                replica_groups=[list(range(n_cores))],
                ins=[cc_in.opt()],
                outs=[cc_out.opt()],
            )
            stats_tot = cpool.tile([f, 2], F32)
            nc.sync.dma_start(stats_tot[:], cc_out[:])

            # --- mean/var -> scale/shift ---
            mean = sc_sb[:, 0:1]
            es2 = sc_sb[:, 1:2]
            msq = sc_sb[:, 2:3]
            var = sc_sb[:, 3:4]
            std = sc_sb[:, 4:5]
            rstd = sc_sb[:, 5:6]
            scl = sc_sb[:, 6:7]
            tmp = sc_sb[:, 7:8]
            shf = sc_sb[:, 8:9]
            nc.vector.tensor_scalar_mul(mean, stats_tot[:, 0:1], inv_count)
            nc.vector.tensor_scalar_mul(es2, stats_tot[:, 1:2], inv_count)
            nc.vector.tensor_mul(msq, mean, mean)
            nc.vector.tensor_sub(var, es2, msq)
            varp = sc_sb[:, 9:10]
            nc.vector.tensor_scalar_add(varp, var, BN_EPS)
            nc.scalar.activation(std, varp, mybir.ActivationFunctionType.Sqrt)
            nc.vector.reciprocal(rstd, std)
            nc.vector.tensor_mul(scl, gamma_sb[:], rstd)
            nc.vector.tensor_mul(tmp, mean, scl)
            nc.vector.tensor_sub(shf, beta_sb[:], tmp)

            # --- apply BN+ReLU, untranspose, store ---
            for b in range(b_pc):
                zr_sb = iopool.tile([f, n], F32, tag="zr")
                nc.scalar.activation(
                    zr_sb[:],
                    zq_tiles[b][:],
                    mybir.ActivationFunctionType.Relu,
                    bias=shf,
                    scale=scl,
                )
                out_sb = iopool.tile([128, NT, f], F32, tag="out")
                for c in range(NT):
                    bpsum = ppool.tile([128, f], F32, tag="bpsum")
                    nc.tensor.transpose(
                        bpsum[:], zr_sb[:, c * 128 : (c + 1) * 128], i64_sb[:]
                    )
                    nc.vector.tensor_copy(out_sb[:, c, :], bpsum[:])
                nc.sync.dma_start(
                    out[b].rearrange("(t p) f -> p t f", p=128), out_sb[:]
                )

    nc.compile()
    return nc


def make_in_maps(node_feats, edge_feats, W, gamma, beta, n_cores=N_CORES):
    b, n, f = node_feats.shape
    b_pc = b // n_cores
    node_feats = np.asarray(node_feats, dtype=np.float32)
    edge_feats = np.asarray(edge_feats, dtype=np.float32)
    wt = np.ascontiguousarray(np.asarray(W, dtype=np.float32).T)
    gamma = np.asarray(gamma, dtype=np.float32).reshape(f, 1)
    beta = np.asarray(beta, dtype=np.float32).reshape(f, 1)
    i128 = np.eye(128, dtype=np.float32).astype(ml_dtypes.bfloat16)
    i64 = np.eye(f, dtype=np.float32)
    in_maps = []
    for c in range(n_cores):
        sl = slice(c * b_pc, (c + 1) * b_pc)
        xs = node_feats[sl]
        in_maps.append(
            {
                "edge": edge_feats[sl],
                "xt": np.ascontiguousarray(xs.transpose(0, 2, 1)),
                "xn": (xs / np.float32(n)).astype(ml_dtypes.bfloat16),
                "wt": wt,
                "gamma": gamma,
                "beta": beta,
                "i128": i128,
                "i64": i64,
            }
        )
    return in_maps


_NC_CACHE = {}


def _get_nc(key=(N_CORES, B_PC, N, F)):
    if key not in _NC_CACHE:
        _NC_CACHE[key] = build_nc(*key)
    return _NC_CACHE[key]


def kernel(node_feats, edge_feats, W, gamma, beta):
    node_feats = np.asarray(node_feats)
    edge_feats = np.asarray(edge_feats)
    b, n, f = node_feats.shape
    n_cores = N_CORES
    b_pc = b // n_cores
    nc = _get_nc((n_cores, b_pc, n, f))
    in_maps = make_in_maps(node_feats, edge_feats, W, gamma, beta, n_cores)
    res = run_bass_kernel_spmd(nc, in_maps, list(range(n_cores)))
    outs = [res.results[c]["out"] for c in range(n_cores)]
    return np.concatenate(outs, axis=0).astype(np.float32)


# revision 8
# speedup vs baseline: 9.0621x; 9.0621x over previous
"""Trainium2 Bass kernel for NodeUpdateNetwork-style GNN message passing.

out = relu(BN((x + ((sim - dsim) @ x) / N) @ W.T))  with sync-BN over (B, N).

Sharding: data-parallel over batch across 8 NeuronCores (2 batches/core);
W/gamma/beta replicated; BN statistics all-reduced across cores in-kernel.

The on-chip pipeline keeps the feature dimension on partitions ("transposed
space") so that BN reduces run along the free axis and the BN+ReLU apply is a
single per-partition scalar-engine activation:
  - stream sim/dsim row-stripes [128, N] fp32 (contiguous HBM reads)
  - DVE: diff = sim - dsim (bf16 out)
  - PE: transpose 128x128 diff tiles (identity matmul, bf16)
  - PE: aggT[f, i] += (x/N)[j, f]^T-contracted with diffT[j, i]
  - yT = aggT + xT ; zT = W @ yT (fp32) ; BN stats; AllReduce; apply; untranspose
"""

import sys

if "/opt/trn_rl_repo" not in sys.path:
    sys.path.insert(0, "/opt/trn_rl_repo")

import numpy as np
import ml_dtypes

import concourse.bacc as bacc
import concourse.mybir as mybir
import concourse.tile as tile
from concourse.bass_utils import run_bass_kernel_spmd

N_CORES = 8
B, N, F = 16, 2048, 64
B_PC = B // N_CORES
BN_EPS = 1e-5
BF16 = mybir.dt.bfloat16
F32 = mybir.dt.float32


def build_nc(n_cores=N_CORES, b_pc=B_PC, n=N, f=F, b_total=None, reps=1):
    """Build the per-core Bass program (same program on every core).

    reps > 1 unrolls the whole computation multiple times (for timing-slope
    measurements: HW time per pass = (t(reps=R) - t(reps=1)) / (R - 1)).
    """
    assert f == 64
    if b_total is None:
        b_total = n_cores * b_pc
    NT = n // 128                      # number of 128-wide j tiles
    IB = min(4, NT)                    # i-blocks (128 rows) per group
    GW = IB * 128                      # group width along i (<= 512)
    NG = n // GW                       # groups per batch
    inv_count = 1.0 / (b_total * n)

    nc = bacc.Bacc(
        "TRN2", target_bir_lowering=False, debug=False, num_devices=n_cores
    )

    edge = nc.dram_tensor("edge", [b_pc, 2, n, n], F32, kind="ExternalInput").ap()
    xt = nc.dram_tensor("xt", [b_pc, f, n], F32, kind="ExternalInput").ap()
    xn = nc.dram_tensor("xn", [b_pc, n, f], BF16, kind="ExternalInput").ap()
    wt = nc.dram_tensor("wt", [f, f], F32, kind="ExternalInput").ap()
    gamma = nc.dram_tensor("gamma", [f, 1], F32, kind="ExternalInput").ap()
    beta = nc.dram_tensor("beta", [f, 1], F32, kind="ExternalInput").ap()
    i128 = nc.dram_tensor("i128", [128, 128], BF16, kind="ExternalInput").ap()
    i64 = nc.dram_tensor("i64", [f, f], F32, kind="ExternalInput").ap()
    out = nc.dram_tensor("out", [b_pc, n, f], F32, kind="ExternalOutput").ap()

    with tile.TileContext(nc) as tc:
        with (
            tc.tile_pool(name="const", bufs=1) as cpool,
            tc.tile_pool(name="io", bufs=2) as iopool,
            tc.tile_pool(name="zq", bufs=b_pc) as zqpool,
            tc.tile_pool(name="stream", bufs=3) as spool,
            tc.tile_pool(name="blk", bufs=2) as bpool,
            tc.tile_pool(name="psum", bufs=2, space="PSUM") as ppool,
            tc.tile_pool(name="dram", bufs=2, space="DRAM") as dpool,
        ):
            # --- constants ---
            i128_sb = cpool.tile([128, 128], BF16)
            nc.sync.dma_start(i128_sb[:], i128[:])
            i64_sb = cpool.tile([f, f], F32)
            nc.sync.dma_start(i64_sb[:], i64[:])
            wt_sb = cpool.tile([f, f], F32)
            nc.sync.dma_start(wt_sb[:], wt[:])
            gamma_sb = cpool.tile([f, 1], F32)
            nc.sync.dma_start(gamma_sb[:], gamma[:])
            beta_sb = cpool.tile([f, 1], F32)
            nc.sync.dma_start(beta_sb[:], beta[:])
            stats_sb = cpool.tile([f, b_pc * NG, 2], F32)
            sc_sb = cpool.tile([f, 12], F32)

            def one_pass():
                zq_tiles = []
                for b in range(b_pc):
                    # --- per-batch node features ---
                    xt_sb = iopool.tile([f, n], F32, tag="xt")
                    nc.sync.dma_start(xt_sb[:], xt[b])
                    xn_sb = iopool.tile([128, NT, f], BF16, tag="xn")
                    nc.sync.dma_start(
                        xn_sb[:], xn[b].rearrange("(t p) f -> p t f", p=128)
                    )
                    zq_sb = zqpool.tile([f, n], F32, tag="zq")
                    zq_tiles.append(zq_sb)

                    for g in range(NG):
                        # --- load group stripes and subtract ---
                        diff_all = bpool.tile([128, IB, n], BF16, tag="diff")
                        for ib in range(IB):
                            i0 = g * GW + ib * 128
                            sim_sb = spool.tile([128, n], F32, tag="sim")
                            nc.sync.dma_start(
                                sim_sb[:], edge[b, 0, i0 : i0 + 128, :]
                            )
                            dsim_sb = spool.tile([128, n], F32, tag="dsim")
                            nc.sync.dma_start(
                                dsim_sb[:], edge[b, 1, i0 : i0 + 128, :]
                            )
                            nc.vector.tensor_sub(
                                diff_all[:, ib, :], sim_sb[:], dsim_sb[:]
                            )

                        # --- transpose diff tiles: dT[j, i] = diff[i, j] ---
                        dT_all = bpool.tile([128, NT, GW], BF16, tag="dT")
                        for jt in range(NT):
                            tpsum = ppool.tile([128, GW], BF16, tag="tpsum")
                            for ib in range(IB):
                                nc.tensor.transpose(
                                    tpsum[:, ib * 128 : (ib + 1) * 128],
                                    diff_all[:, ib, jt * 128 : (jt + 1) * 128],
                                    i128_sb[:],
                                )
                            if jt % 2 == 0:
                                nc.vector.tensor_copy(dT_all[:, jt, :], tpsum[:])
                            else:
                                nc.scalar.copy(dT_all[:, jt, :], tpsum[:])

                        # --- aggT[f, i] = sum_j (x/N)[j, f] * diff[i, j] ---
                        aggT = ppool.tile([f, GW], F32, tag="agg")
                        for jt in range(NT):
                            nc.tensor.matmul(
                                aggT[:],
                                xn_sb[:, jt, :],
                                dT_all[:, jt, :],
                                start=(jt == 0),
                                stop=(jt == NT - 1),
                            )

                        # --- yT = aggT + xT ; zT = W @ yT ---
                        yT_sb = bpool.tile([f, GW], F32, tag="yT")
                        nc.vector.tensor_add(
                            yT_sb[:], aggT[:], xt_sb[:, g * GW : (g + 1) * GW]
                        )
                        zT = ppool.tile([f, GW], F32, tag="zT")
                        nc.tensor.matmul(
                            zT[:], wt_sb[:], yT_sb[:], start=True, stop=True
                        )

                        # stash z and accumulate BN partial sums
                        nc.scalar.copy(zq_sb[:, g * GW : (g + 1) * GW], zT[:])
                        gi = b * NG + g
                        nc.vector.reduce_sum(
                            stats_sb[:, gi, 0:1], zT[:], axis=mybir.AxisListType.X
                        )
                        sq_sb = bpool.tile([f, GW], F32, tag="sq")
                        nc.scalar.activation(
                            sq_sb[:],
                            zT[:],
                            mybir.ActivationFunctionType.Square,
                            accum_out=stats_sb[:, gi, 1:2],
                        )

                # --- local stats -> global stats (sync-BN all-reduce) ---
                stats_loc = cpool.tile([f, 2], F32, tag="stats_loc")
                nc.vector.reduce_sum(
                    stats_loc[:],
                    stats_sb[:].rearrange("p g s -> p s g"),
                    axis=mybir.AxisListType.X,
                )
                cc_in = dpool.tile([f, 2], F32, tag="cc_in")
                cc_out = dpool.tile([f, 2], F32, tag="cc_out")
                nc.sync.dma_start(cc_in[:], stats_loc[:])
                nc.gpsimd.collective_compute(
                    "AllReduce",
                    mybir.AluOpType.add,
                    replica_groups=[list(range(n_cores))],
                    ins=[cc_in.opt()],
                    outs=[cc_out.opt()],
                )
                stats_tot = cpool.tile([f, 2], F32, tag="stats_tot")
                nc.sync.dma_start(stats_tot[:], cc_out[:])

                # --- mean/var -> scale/shift ---
                mean = sc_sb[:, 0:1]
                es2 = sc_sb[:, 1:2]
                msq = sc_sb[:, 2:3]
                var = sc_sb[:, 3:4]
                std = sc_sb[:, 4:5]
                rstd = sc_sb[:, 5:6]
                scl = sc_sb[:, 6:7]
                tmp = sc_sb[:, 7:8]
                shf = sc_sb[:, 8:9]
                varp = sc_sb[:, 9:10]
                nc.vector.tensor_scalar_mul(mean, stats_tot[:, 0:1], inv_count)
                nc.vector.tensor_scalar_mul(es2, stats_tot[:, 1:2], inv_count)
                nc.vector.tensor_mul(msq, mean, mean)
                nc.vector.tensor_sub(var, es2, msq)
                nc.vector.tensor_scalar_add(varp, var, BN_EPS)
                nc.scalar.activation(std, varp, mybir.ActivationFunctionType.Sqrt)
                nc.vector.reciprocal(rstd, std)
                nc.vector.tensor_mul(scl, gamma_sb[:], rstd)
                nc.vector.tensor_mul(tmp, mean, scl)
                nc.vector.tensor_sub(shf, beta_sb[:], tmp)

                # --- apply BN+ReLU, untranspose, store ---
                for b in range(b_pc):
                    zr_sb = iopool.tile([f, n], F32, tag="zr")
                    nc.scalar.activation(
                        zr_sb[:],
                        zq_tiles[b][:],
                        mybir.ActivationFunctionType.Relu,
                        bias=shf,
                        scale=scl,
                    )
                    out_sb = iopool.tile([128, NT, f], F32, tag="out")
                    for c in range(NT):
                        bpsum = ppool.tile([128, f], F32, tag="bpsum")
                        nc.tensor.transpose(
                            bpsum[:], zr_sb[:, c * 128 : (c + 1) * 128], i64_sb[:]
                        )
                        nc.vector.tensor_copy(out_sb[:, c, :], bpsum[:])
                    nc.sync.dma_start(
                        out[b].rearrange("(t p) f -> p t f", p=128), out_sb[:]
                    )

            for _ in range(reps):
                one_pass()

    nc.compile()
    return nc


def make_in_maps(node_feats, edge_feats, W, gamma, beta, n_cores=N_CORES):
    b, n, f = node_feats.shape
    b_pc = b // n_cores
    node_feats = np.asarray(node_feats, dtype=np.float32)
    edge_feats = np.asarray(edge_feats, dtype=np.float32)
    wt = np.ascontiguousarray(np.asarray(W, dtype=np.float32).T)
    gamma = np.asarray(gamma, dtype=np.float32).reshape(f, 1)
    beta = np.asarray(beta, dtype=np.float32).reshape(f, 1)
    i128 = np.eye(128, dtype=np.float32).astype(ml_dtypes.bfloat16)
    i64 = np.eye(f, dtype=np.float32)
    in_maps = []
    for c in range(n_cores):
        sl = slice(c * b_pc, (c + 1) * b_pc)
        xs = node_feats[sl]
        in_maps.append(
            {
                "edge": edge_feats[sl],
                "xt": np.ascontiguousarray(xs.transpose(0, 2, 1)),
                "xn": (xs / np.float32(n)).astype(ml_dtypes.bfloat16),
                "wt": wt,
                "gamma": gamma,
                "beta": beta,
                "i128": i128,
                "i64": i64,
            }
        )
    return in_maps


_NC_CACHE = {}


def _get_nc(key=(N_CORES, B_PC, N, F)):
    if key not in _NC_CACHE:
        _NC_CACHE[key] = build_nc(*key)
    return _NC_CACHE[key]


def kernel(node_feats, edge_feats, W, gamma, beta):
    node_feats = np.asarray(node_feats)
    edge_feats = np.asarray(edge_feats)
    b, n, f = node_feats.shape
    n_cores = N_CORES
    b_pc = b // n_cores
    nc = _get_nc((n_cores, b_pc, n, f))
    in_maps = make_in_maps(node_feats, edge_feats, W, gamma, beta, n_cores)
    res = run_bass_kernel_spmd(nc, in_maps, list(range(n_cores)))
    outs = [res.results[c]["out"] for c in range(n_cores)]
    return np.concatenate(outs, axis=0).astype(np.float32)
